# revision 9
# baseline (speedup 1.0000x reference)
"""Continuous-filter convolution (SchNet-style) Trainium2 kernel.

Sharding: data-parallel by graph. 8 graphs x 2500 nodes; core g owns graph g
(nodes [2500g, 2500(g+1))), so gather / filter MLP / scatter are fully local.

Per-core device pipeline (edges sorted by dest node):
  1. dma_gather: nfc[src]  -> [128, C, 320]  (node_feats | coords | pad)
     dma_gather: cd4[dest] -> [128, C, 64]   (coords | pad)
  2. d = |c_src - c_dst|   (DVE sub/mul, reduce; ACT sqrt)
  3. rbf[e,b] = exp(-g*(d-c_b)^2)  (ACT Square with per-partition bias, Exp)
  4. PE transpose rbf -> rbfT [64, 512]; mm1: hT = relu(W1T.T @ rbfT)
  5. mm2: M = relu(hT.T @ W2T)  [128e, 256]
  6. msgs = nf[src] * M        (DVE)
  7. scatter-add: one-hot S[e, v] = (iota == dest_rel) built on DVE;
     acc[vtile] += S.T @ msgs accumulated in PSUM; copy out per 128-node tile.

Matmuls run in float32r (full-rate PE; ~tf32 precision).
"""

import math
import sys

import numpy as np

kernel = sys.modules[__name__]
last_results = None

P = 128
N_GRAPHS = 8
V_PER = 2500
HIDDEN = 256
NB = 64
VPAD = 2560          # V_PER rounded up to 20 tiles of 128
NVT = VPAD // P      # 20 dest tiles
ZROW = VPAD - 1      # all-zero row used by padding edges (node 2559 >= 2500)
NFC_W = 320          # 256 feats + 3 coords + pad -> 1280B rows (%256 == 0)
CD_W = 64            # 3 coords + pad -> 256B rows
CPG = 16             # chunks (128 edges) per gather group (2048 edges)

_CENTERS = np.linspace(0.0, 1.0, NB, dtype=np.float32)
_GAMMA = float(1.0 / (_CENTERS[1] - _CENTERS[0]) ** 2)


def _wrap_idx(flat_idx: np.ndarray) -> np.ndarray:
    """int16 [128, n/16] dma_gather index layout: idx[ch, k] = flat[16k + ch],
    replicated across the 8 q7 cores (partition stripes of 16)."""
    n = flat_idx.shape[0]
    assert n % 16 == 0
    w = flat_idx.astype(np.int16).reshape(n // 16, 16).T  # [16, n/16]
    return np.tile(w, (8, 1))                             # [128, n/16]


def _prep_core(src_l, dest_l, chunks_per_tile):
    """Order/pad one core's edges to the shared schedule.

    Returns (sidx, didx, drel) flat arrays of length 128*sum(chunks_per_tile),
    edge slot j -> chunk j//128, partition j%128.
    """
    order = np.argsort(dest_l, kind="stable")
    src_l, dest_l = src_l[order], dest_l[order]
    tile_of = dest_l // P
    e_pad = P * int(sum(chunks_per_tile))
    sidx = np.full(e_pad, ZROW, np.int32)
    didx = np.full(e_pad, ZROW, np.int32)
    drel = np.zeros(e_pad, np.float32)
    pos = 0
    start = 0
    for t in range(NVT):
        cnt = int(np.searchsorted(tile_of, t + 1) - start)
        sl = slice(start, start + cnt)
        sidx[pos:pos + cnt] = src_l[sl]
        didx[pos:pos + cnt] = dest_l[sl]
        drel[pos:pos + cnt] = (dest_l[sl] - P * t).astype(np.float32)
        start += cnt
        pos += P * int(chunks_per_tile[t])
    return sidx, didx, drel


def _schedule(chunks_per_tile):
    """Per 128-edge chunk: (vtile, is_first, is_last)."""
    sched = []
    for t in range(NVT):
        n = int(chunks_per_tile[t])
        for k in range(n):
            sched.append((t, k == 0, k == n - 1))
    return sched


def _build_program(sched):
    import concourse.bass as bass
    import concourse.tile as tile
    from concourse import bacc, mybir

    f32 = mybir.dt.float32
    f32r = mybir.dt.float32r
    i16 = mybir.dt.int16
    AF = mybir.ActivationFunctionType
    ALU = mybir.AluOpType

    n_chunks = len(sched)
    assert n_chunks % CPG == 0
    n_groups = n_chunks // CPG          # 2048-edge gather groups
    ec = n_chunks * P // 16             # idx columns

    nc = bacc.Bacc("TRN2", target_bir_lowering=False, debug=False,
                   num_devices=N_GRAPHS)

    nfc_d = nc.dram_tensor("nfc", [VPAD, NFC_W], f32, kind="ExternalInput")
    cd4_d = nc.dram_tensor("cd4", [VPAD, CD_W], f32, kind="ExternalInput")
    w1t_d = nc.dram_tensor("w1t", [NB, HIDDEN], f32r, kind="ExternalInput")
    w2t_d = nc.dram_tensor("w2t", [HIDDEN, HIDDEN], f32r, kind="ExternalInput")
    cb_d = nc.dram_tensor("cb", [P, NB], f32, kind="ExternalInput")
    iov_d = nc.dram_tensor("iov", [P, P], f32, kind="ExternalInput")
    idm_d = nc.dram_tensor("idm", [P, P], f32, kind="ExternalInput")
    drel_d = nc.dram_tensor("drel", [P, n_chunks], f32, kind="ExternalInput")
    sidx_d = nc.dram_tensor("sidx", [P, ec], i16, kind="ExternalInput")
    didx_d = nc.dram_tensor("didx", [P, ec], i16, kind="ExternalInput")
    hout_d = nc.dram_tensor("hout", [VPAD, HIDDEN], f32, kind="ExternalOutput")

    with tile.TileContext(nc) as tc:
        with (
            tc.tile_pool(name="const", bufs=1) as cpool,
            tc.tile_pool(name="gather", bufs=3) as gpool,
            tc.tile_pool(name="work", bufs=2) as wpool,
            tc.tile_pool(name="psum", bufs=1, space="PSUM") as ppool,
        ):
            # ---- constants ----
            w1t_sb = cpool.tile([NB, HIDDEN], f32r)
            nc.sync.dma_start(w1t_sb[:], w1t_d[:, :])
            w2t_sb = cpool.tile([P, 2, HIDDEN], f32r)
            nc.sync.dma_start(w2t_sb[:, 0, :], w2t_d[0:P, :])
            nc.sync.dma_start(w2t_sb[:, 1, :], w2t_d[P:2 * P, :])
            cb_sb = cpool.tile([P, NB], f32)
            nc.sync.dma_start(cb_sb[:], cb_d[:, :])
            iov_sb = cpool.tile([P, P], f32)
            nc.sync.dma_start(iov_sb[:], iov_d[:, :])
            drel_sb = cpool.tile([P, n_chunks], f32)
            nc.sync.dma_start(drel_sb[:], drel_d[:, :])
            sidx_sb = cpool.tile([P, ec], i16)
            nc.sync.dma_start(sidx_sb[:], sidx_d[:, :])
            didx_sb = cpool.tile([P, ec], i16)
            nc.sync.dma_start(didx_sb[:], didx_d[:, :])
            ident = cpool.tile([P, P], f32)
            nc.sync.dma_start(ident[:], idm_d[:, :])

            acc_ps = None
            for gi in range(n_groups):
                c0 = gi * CPG
                # ---- gathers (2048 edges) ----
                icols = slice(gi * P, (gi + 1) * P)
                nfc_g = gpool.tile([P, CPG, NFC_W], f32, tag="nfc")
                cd_g = gpool.tile([P, CPG, CD_W], f32, tag="cd")
                half = CPG * P // 2
                for hh in range(2):
                    hcols = slice(gi * P + hh * 64, gi * P + (hh + 1) * 64)
                    csl = slice(hh * CPG // 2, (hh + 1) * CPG // 2)
                    nc.gpsimd.dma_gather(
                        nfc_g[:, csl, :], nfc_d[:, :], sidx_sb[:, hcols],
                        num_idxs=half, num_idxs_reg=half, elem_size=NFC_W)
                    nc.gpsimd.dma_gather(
                        cd_g[:, csl, :], cd4_d[:, :], didx_sb[:, hcols],
                        num_idxs=half, num_idxs_reg=half, elem_size=CD_W)

                # ---- distances for the 16 chunks ----
                dif = wpool.tile([P, CPG, 4], f32, tag="dif")
                nc.vector.tensor_tensor(
                    dif[:, :, 0:3], nfc_g[:, :, 256:259], cd_g[:, :, 0:3],
                    op=ALU.subtract)
                sq = wpool.tile([P, CPG, 4], f32, tag="sq")
                nc.vector.tensor_tensor(
                    sq[:, :, 0:3], dif[:, :, 0:3], dif[:, :, 0:3], op=ALU.mult)
                d2 = wpool.tile([P, CPG], f32, tag="d2")
                nc.vector.tensor_reduce(
                    d2[:], sq[:, :, 0:3], axis=mybir.AxisListType.X,
                    op=ALU.add)
                dd = wpool.tile([P, CPG], f32, tag="dd")
                nc.scalar.sqrt(dd[:], d2[:])
                negd = wpool.tile([P, CPG], f32, tag="negd")
                nc.vector.tensor_scalar_mul(negd[:], dd[:], -1.0)

                for sg in range(CPG // 4):      # 512-edge mm1 groups
                    rbfT_ps = ppool.tile([NB, 4 * P], f32, tag="rbfT",
                                         space="PSUM")
                    for s4 in range(4):
                        s = sg * 4 + s4
                        rbf_e = wpool.tile([P, NB], f32, tag="rbf", bufs=3)
                        nc.scalar.activation(
                            rbf_e[:], cb_sb[:], AF.Square,
                            bias=negd[:, s:s + 1])
                        rbf2 = wpool.tile([P, NB], f32, tag="rbf2", bufs=3)
                        nc.scalar.activation(
                            rbf2[:], rbf_e[:], AF.Exp, scale=-_GAMMA)
                        nc.tensor.transpose(
                            rbfT_ps[:, s4 * P:(s4 + 1) * P], rbf2[:],
                            ident[:])
                    rbfT_sb = wpool.tile([NB, 4 * P], f32r, tag="rbfT_sb")
                    nc.scalar.copy(rbfT_sb[:], rbfT_ps[:])

                    hT_ps0 = ppool.tile([P, 4 * P], f32, tag="hT", bufs=3,
                                        space="PSUM")
                    hT_ps1 = ppool.tile([P, 4 * P], f32, tag="hT", bufs=3,
                                        space="PSUM")
                    nc.tensor.matmul(
                        hT_ps0[:], lhsT=w1t_sb[:, 0:P],
                        rhs=rbfT_sb[:], start=True, stop=True)
                    nc.tensor.matmul(
                        hT_ps1[:], lhsT=w1t_sb[:, P:2 * P],
                        rhs=rbfT_sb[:], start=True, stop=True)
                    hT_sb = wpool.tile([P, 2, 4 * P], f32r, tag="hT_sb")
                    nc.scalar.activation(hT_sb[:, 0, :], hT_ps0[:], AF.Relu)
                    nc.scalar.activation(hT_sb[:, 1, :], hT_ps1[:], AF.Relu)

                    for s4 in range(4):
                        s = sg * 4 + s4
                        c = c0 + s
                        m_ps = ppool.tile([P, HIDDEN], f32, tag="m", bufs=2,
                                          space="PSUM")
                        esl = slice(s4 * P, (s4 + 1) * P)
                        nc.tensor.matmul(
                            m_ps[:], lhsT=hT_sb[:, 0, esl],
                            rhs=w2t_sb[:, 0, :],
                            start=True, stop=False)
                        nc.tensor.matmul(
                            m_ps[:], lhsT=hT_sb[:, 1, esl],
                            rhs=w2t_sb[:, 1, :],
                            start=False, stop=True)
                        mr = wpool.tile([P, HIDDEN], f32, tag="mr", bufs=3)
                        if s4 % 2 == 0:
                            nc.vector.tensor_scalar_max(mr[:], m_ps[:], 0.0)
                        else:
                            nc.scalar.activation(mr[:], m_ps[:], AF.Relu)
                        msgs = wpool.tile([P, HIDDEN], f32r, tag="msgs",
                                          bufs=3)
                        nc.vector.tensor_tensor(
                            msgs[:], mr[:], nfc_g[:, s, 0:HIDDEN],
                            op=ALU.mult)
                        s_sb = wpool.tile([P, P], f32r, tag="S", bufs=3)
                        nc.gpsimd.tensor_scalar(
                            s_sb[:], iov_sb[:], drel_sb[:, c:c + 1], None,
                            op0=ALU.is_equal)

                        t, first, last = sched[c]
                        if first:
                            acc_ps = ppool.tile([P, HIDDEN], f32, tag="acc",
                                                bufs=2, space="PSUM")
                        nc.tensor.matmul(
                            acc_ps[:], lhsT=s_sb[:],
                            rhs=msgs[:],
                            start=first, stop=last, skip_group_check=True)
                        if last:
                            ho = wpool.tile([P, HIDDEN], f32, tag="ho")
                            nc.vector.tensor_copy(ho[:], acc_ps[:])
                            nc.sync.dma_start(
                                hout_d[t * P:(t + 1) * P, :], ho[:])

    nc.compile()
    return nc


def kernel(node_feats, coords, W1, W2, batch_index, src, dest):
    from concourse.bass_utils import run_bass_kernel_spmd

    node_feats = np.asarray(node_feats, np.float32)
    coords = np.asarray(coords, np.float32)
    W1 = np.asarray(W1, np.float32)
    W2 = np.asarray(W2, np.float32)
    src = np.asarray(src, np.int64)
    dest = np.asarray(dest, np.int64)

    # ---- shard by graph, shared (max-over-cores) chunk schedule ----
    per_core = []
    needs = np.zeros((N_GRAPHS, NVT), np.int64)
    for g in range(N_GRAPHS):
        lo, hi = g * V_PER, (g + 1) * V_PER
        m = (dest >= lo) & (dest < hi)
        s_l = (src[m] - lo).astype(np.int32)
        d_l = (dest[m] - lo).astype(np.int32)
        per_core.append((s_l, d_l))
        needs[g] = np.bincount(
            np.minimum(d_l // P, NVT - 1),
            minlength=NVT)
    chunks_per_tile = [
        int(math.ceil(int(needs[:, t].max()) / P)) for t in range(NVT)
    ]
    # keep every tile live and total a multiple of CPG
    chunks_per_tile = [max(c, 1) for c in chunks_per_tile]
    extra = (-sum(chunks_per_tile)) % CPG
    chunks_per_tile[-1] += extra
    sched = _schedule(chunks_per_tile)
    n_chunks = len(sched)

    # ---- per-core input tensors ----
    in_maps = []
    for g in range(N_GRAPHS):
        lo, hi = g * V_PER, (g + 1) * V_PER
        s_l, d_l = per_core[g]
        sidx, didx, drel = _prep_core(s_l, d_l, chunks_per_tile)
        nfc = np.zeros((VPAD, NFC_W), np.float32)
        nfc[:V_PER, :HIDDEN] = node_feats[lo:hi]
        nfc[:V_PER, HIDDEN:HIDDEN + 3] = coords[lo:hi]
        cd4 = np.zeros((VPAD, CD_W), np.float32)
        cd4[:V_PER, 0:3] = coords[lo:hi]
        in_maps.append({
            "nfc": nfc,
            "cd4": cd4,
            "w1t": np.ascontiguousarray(W1.T),
            "w2t": np.ascontiguousarray(W2.T),
            "cb": np.tile(_CENTERS, (P, 1)),
            "iov": np.tile(np.arange(P, dtype=np.float32), (P, 1)),
            "idm": np.eye(P, dtype=np.float32),
            "drel": np.ascontiguousarray(
                drel.reshape(n_chunks, P).T),
            "sidx": _wrap_idx(sidx),
            "didx": _wrap_idx(didx),
        })

    nc = _build_program(sched)
    import os
    trace = bool(os.environ.get("BASS_KERNEL_TRACE"))
    res = run_bass_kernel_spmd(nc, in_maps, core_ids=list(range(N_GRAPHS)),
                               trace=trace)
    if trace:
        print("HW exec time:", res.exec_time_ns, "ns")
        kernel.last_results = res

    out = np.empty((N_GRAPHS * V_PER, HIDDEN), np.float32)
    for g in range(N_GRAPHS):
        out[g * V_PER:(g + 1) * V_PER] = res.results[g]["hout"][:V_PER]
    return out
